# revision 10
# baseline (speedup 1.0000x reference)
"""Trainium2 Bass kernel for nn_ContrastiveCRFLoss (self-contained).

Math: for each batch b and sample pair (n, m) over 2048 gathered pixels:
    out[b,n,m] = -(C[b,n,m] * (W1*exp(-cd - gd[b]/(2*BETA)) + W2*exp(-cd/(2*GAMMA))))
where C = cluster Gram, cd = squared coord distance, gd = squared guidance
distance.  The output is SYMMETRIC in (n, m): C is a Gram matrix and both
exp kernels depend on symmetric distances.

Device strategy (8 cores, upper-triangle only, mirrored on host):
  - The 2048x2048 pair grid is cut into 16 row-blocks of 128.  Block i needs
    only columns [128*i, 2048) (upper triangle incl. the diagonal block).
    Core k owns blocks k and 15-k: (16-k)+(k+1) = 17 col-units of 128 ->
    exactly 8 tiles of 256 cols + 1 tile of 128 cols per batch on EVERY
    core (SPMD-uniform shapes; per-core geometry lives in host staging).
  - Three fp16 matmul streams per tile, spread over all four PE row groups
    (tile_position 0/32/64/96) so independent streams run concurrently:
      even batches: Gram at rows 0,  exp1-arg at rows 32
      odd  batches: Gram at rows 64, exp1-arg at rows 96
      exp2-arg (batch-independent): rows 0 / 96 by chunk
    Exp arguments are produced directly in PSUM by augmenting the operands
    with ones/norm/log-weight rows (hi/lo fp16 splits keep this exact).
  - Inputs are packed so each parity loads with ONE [41 x cols] DMA (wc at
    group base, a1 at group base+32) - 6 input DMA triggers total.
  - Per 512-col chunk: ACT exp(PSUM)->fp16, add e1+e2 split GpSimd/DVE;
    DVE mult over 1024-col pC superchunks; one [128,2176] store per batch.
  - Host mirrors the strict upper triangle to reconstruct the full output.
"""

import numpy as np

import concourse.bass as bass
import concourse.mybir as mybir
import concourse.bass_utils as bass_utils
from concourse.tile import TileContext
from concourse.vector_clock import ScopedClock

F16 = mybir.dt.float16
F32 = mybir.dt.float32

# problem constants (hardcoded per the task contract)
ALPHA, BETA, GAMMA = 0.5, 0.15, 25.0
W1, W2, SHIFT = 10.0, 3.0, 0.0
B = 8
NS = 2048
NCORES = 8
KC, K1, K2 = 27, 9, 12
NT = 9                       # column tiles per batch per core
TW = [256] * 8 + [128]       # tile widths
# tile -> column offset in the 2176-wide per-batch strip.  Tiles 0-3 then
# tile 8 sit in the first 1152 cols (e2 group 0), tiles 4-7 in the last
# 1024 (e2 group 96).
OFFS = [0, 256, 512, 768, 1152, 1408, 1664, 1920, 1024]
CORE_COLS = 2176
CHUNKS = [(0, 512), (512, 512), (1024, 128), (1152, 512), (1664, 512)]
CHUNK_TILES = [(0, 1), (2, 3), (8,), (4, 5), (6, 7)]
G2E = [0, 0, 0, 96, 96]      # e2-phase row group by chunk
# pC/mult superchunks: (col offset, width, chunk indices)
SUPER = [(0, 1024, (0, 1)), (1024, 128, (2,)), (1152, 1024, (3, 4))]
# ge (e2 operand pack) column bookkeeping
WE_COL = {0: 0, 1: 128, 2: 256, 3: 384, 8: 512, 4: 1792, 5: 1920, 6: 2048, 7: 2176}
RE_BASE = {0: 640, 1: 1152, 2: 1664, 3: 2304, 4: 2816}
GE_COLS = 3328

# ---------------------------------------------------------------------------
# Walrus in this image rejects >1 sync wait per instruction. Split the Tile
# tail-drain's waits and any multi-wait instruction into single-wait NOPs.
# ---------------------------------------------------------------------------
_MAXW = 1


def _split_drain_and_barrier(self, tick_clock, wait_clock):
    probe = self.nc.sync.nop(nofuse=True)
    wait_clock.add_sem_waits(probe.ins, ScopedClock({None: tick_clock.global_clock}))
    si = probe.ins.sync_info
    waits = list(si.on_wait)
    probe.ins.sync_info = mybir.SyncInfo(
        on_wait=waits[:_MAXW], on_update=list(si.on_update)
    )
    for i in range(_MAXW, len(waits), _MAXW):
        n2 = self.nc.sync.nop(nofuse=True)
        n2.ins.sync_info = mybir.SyncInfo(on_wait=waits[i : i + _MAXW], on_update=[])
    self.nc.sync.drain()
    self.nc.all_engine_barrier()
    popped = self.nc._tile_sem_poison_stack.pop()
    assert popped is self._sem_poison
    self.nc.clear_and_free_semaphores(list(self.sems.allocated().values()))
    self.nc.all_engine_barrier()


def _split_multiwait_insts(nc):
    n_split = 0
    for fn in nc.m.functions:
        for bb in fn.blocks:
            insts = list(bb.instructions)
            new_insts = []
            changed = False
            for inst in insts:
                si = inst.sync_info
                waits = list(si.on_wait) if si is not None else []
                if len(waits) > _MAXW:
                    n_split += 1
                    changed = True
                    n_extra = len(waits) - _MAXW
                    for i in range(0, n_extra, _MAXW):
                        nop = mybir.InstNoOp(
                            name=nc.get_next_instruction_name(),
                            engine=inst.engine,
                            bass_nofuse=True,
                            sync_info=mybir.SyncInfo(
                                on_wait=waits[i : i + _MAXW], on_update=[]
                            ),
                        )
                        new_insts.append(nop)
                    inst.sync_info = mybir.SyncInfo(
                        on_wait=waits[n_extra:], on_update=list(si.on_update)
                    )
                new_insts.append(inst)
            if changed:
                bb.instructions = new_insts
    return n_split


def _install_tile_patch():
    TileContext._drain_and_barrier = _split_drain_and_barrier


def _tiles_for_core(k):
    """17 col-units as 8x256 + 1x128 tiles: (row_block, unit_start, n_units)."""
    WA, WB = 16 - k, k + 1
    A, Bb = k, 15 - k
    tiles = []
    for j in range(0, WA - 1, 2):
        tiles.append((A, j, 2))
    for j in range(0, WB - 1, 2):
        tiles.append((Bb, j, 2))
    if WA % 2 == 0:
        tiles.append((Bb, WB - 1, 1))
    else:
        tiles.append((A, WA - 1, 1))
    assert len(tiles) == NT and sum(n for _, _, n in tiles) == 17
    return tiles


# ---------------------------------------------------------------------------
# Device program (identical on all cores; data differs per core)
# ---------------------------------------------------------------------------

def build_nc():
    _install_tile_patch()
    nc = bass.Bass()
    WHALF = 4 * NT * 128   # 4608: per-parity lhsT slot columns
    RHALF = 4 * CORE_COLS  # 8704: per-parity rhs columns
    wae = nc.declare_dram_parameter("wae", [41, WHALF], F16, isOutput=False)
    wao = nc.declare_dram_parameter("wao", [41, WHALF], F16, isOutput=False)
    rae = nc.declare_dram_parameter("rae", [41, RHALF], F16, isOutput=False)
    rao = nc.declare_dram_parameter("rao", [41, RHALF], F16, isOutput=False)
    ge = nc.declare_dram_parameter("ge", [K2, GE_COLS], F16, isOutput=False)
    out = nc.declare_dram_parameter("out", [B, 128, CORE_COLS], F16, isOutput=True)

    with TileContext(nc) as tc:
        with (
            tc.tile_pool(name="w", bufs=1) as wpool,
            tc.tile_pool(name="r", bufs=1) as rpool,
            tc.tile_pool(name="e2p", bufs=1) as e2pool,
            tc.tile_pool(name="sb", bufs=12) as sbpool,
            tc.tile_pool(name="sm", bufs=4) as smpool,
            tc.tile_pool(name="ob", bufs=3) as opool,
            tc.tile_pool(name="psA", bufs=4, space="PSUM") as psa,
            tc.tile_pool(name="psB", bufs=2, space="PSUM") as psb,
        ):
            GE = wpool.tile([128, GE_COLS], F16)
            W = wpool.tile([128, 2 * WHALF], F16)
            R = rpool.tile([128, 2 * RHALF], F16)
            # e2 operands first (e2 phase unblocks earliest)
            nc.sync.dma_start(GE[0:K2, 0:1792], ge[:, 0:1792])
            nc.sync.dma_start(GE[96 : 96 + K2, 1792:GE_COLS], ge[:, 1792:GE_COLS])
            # packed parity loads: wc at group base, a1 at base+32
            nc.sync.dma_start(R[0:41, 0:RHALF], rae[:])
            nc.sync.dma_start(W[0:41, 0:WHALF], wae[:])
            nc.sync.dma_start(R[64:105, RHALF : 2 * RHALF], rao[:])
            nc.sync.dma_start(W[64:105, WHALF : 2 * WHALF], wao[:])

            e2 = e2pool.tile([128, CORE_COLS], F16)

            # --- e2 phase: batch-independent second-exp kernel ---
            for c, (off, wd) in enumerate(CHUNKS):
                g = G2E[c]
                p2 = psa.tile([128, 512], F32, tag="pA", name=f"p2c{c}")
                for t in CHUNK_TILES[c]:
                    w = TW[t]
                    toff = OFFS[t] - off
                    nc.tensor.matmul(
                        p2[:, toff : toff + w],
                        GE[g : g + K2, WE_COL[t] : WE_COL[t] + 128],
                        GE[g : g + K2, RE_BASE[c] + toff : RE_BASE[c] + toff + w],
                        start=True,
                        stop=True,
                        tile_position=(g, 0),
                    )
                nc.scalar.activation(
                    e2[:, off : off + wd],
                    p2[:, 0:wd],
                    mybir.ActivationFunctionType.Exp,
                )

            # --- batch loop ---
            for b in range(B):
                par = b % 2
                gc = 0 if par == 0 else 64
                g1 = 32 if par == 0 else 96
                wbase = par * WHALF + (b // 2) * NT * 128
                rbase = par * RHALF + (b // 2) * CORE_COLS
                ob = opool.tile([128, CORE_COLS], F16, tag="ob")
                gps_chunks = {0, 1, 2} if par == 0 else {0, 1}
                e1s = {}
                for c, (off, wd) in enumerate(CHUNKS):
                    p1 = psa.tile([128, 512], F32, tag="pA", name=f"p1b{b}c{c}")
                    for t in CHUNK_TILES[c]:
                        w = TW[t]
                        toff = OFFS[t] - off
                        nc.tensor.matmul(
                            p1[:, toff : toff + w],
                            W[g1 : g1 + K1, wbase + t * 128 : wbase + (t + 1) * 128],
                            R[g1 : g1 + K1, rbase + OFFS[t] : rbase + OFFS[t] + w],
                            start=True,
                            stop=True,
                            tile_position=(g1, 0),
                        )
                    e1 = sbpool.tile([128, 512], F16, tag="e1")
                    nc.scalar.activation(
                        e1[:, 0:wd],
                        p1[:, 0:wd],
                        mybir.ActivationFunctionType.Exp,
                    )
                    e1s[c] = e1
                for so, swd, chs in SUPER:
                    pC = psb.tile([128, 1024], F32, tag="pB", name=f"pCb{b}o{so}")
                    s = smpool.tile([128, 1024], F16, tag="s")
                    for c in chs:
                        off, wd = CHUNKS[c]
                        for t in CHUNK_TILES[c]:
                            w = TW[t]
                            toff = OFFS[t] - so
                            nc.tensor.matmul(
                                pC[:, toff : toff + w],
                                W[gc : gc + KC, wbase + t * 128 : wbase + (t + 1) * 128],
                                R[gc : gc + KC, rbase + OFFS[t] : rbase + OFFS[t] + w],
                                start=True,
                                stop=True,
                                tile_position=(gc, 0),
                            )
                        soff = off - so
                        addfn = (
                            nc.gpsimd.tensor_add
                            if c in gps_chunks
                            else nc.vector.tensor_add
                        )
                        addfn(
                            s[:, soff : soff + wd],
                            e1s[c][:, 0:wd],
                            e2[:, off : off + wd],
                        )
                    nc.vector.tensor_tensor(
                        ob[:, so : so + swd],
                        pC[:, 0:swd],
                        s[:, 0:swd],
                        mybir.AluOpType.mult,
                    )
                nc.sync.dma_start(out[b], ob[:])

    _split_multiwait_insts(nc)
    return nc


# ---------------------------------------------------------------------------
# Host-side input prep
# ---------------------------------------------------------------------------

def _f16(x):
    return np.asarray(x, dtype=np.float16)


def _hi_lo(x):
    """Split fp64 vector into two fp16 rows summing to ~x."""
    hi = _f16(x)
    lo = _f16(x - hi.astype(np.float64))
    return hi, lo


def prepare_inputs(guidance, clusters, coords):
    ci = np.asarray(coords[0], dtype=np.int64)
    cj = np.asarray(coords[1], dtype=np.int64)
    sel_g = guidance[:, :, ci, cj].astype(np.float64)  # [B, 3, NS]
    sel_c = clusters[:, :, ci, cj].astype(np.float32)  # [B, 27, NS]

    # --- cluster Gram operands (fp16 snap) ---
    c16 = _f16(sel_c)
    wc_all = -c16  # lhsT (negated -> folds the leading minus)

    # --- first-exp argument operands: arg1 = -cd - gd/(2*beta) + ln(W1) ---
    u16 = _f16(sel_g / np.sqrt(2.0 * BETA))  # [B, 3, NS]
    xc16 = _f16(np.stack([ci, cj]) - 112.0)  # [2, NS] exact
    f1 = (u16.astype(np.float64) ** 2).sum(1) + (
        xc16.astype(np.float64) ** 2
    ).sum(0)  # [B, NS]
    a1_all = np.empty((B, K1, NS), np.float16)
    r1_all = np.empty((B, K1, NS), np.float16)
    ones = np.ones(NS, np.float16)
    for b in range(B):
        b1h, b1l = _hi_lo(np.log(W1) - f1[b])
        f1h, f1l = _hi_lo(f1[b])
        a1_all[b, 0:3] = u16[b]
        a1_all[b, 3:5] = xc16
        a1_all[b, 5] = ones
        a1_all[b, 6] = ones
        a1_all[b, 7] = f1h
        a1_all[b, 8] = f1l
        r1_all[b, 0:3] = _f16(2.0 * u16[b].astype(np.float64))
        r1_all[b, 3:5] = _f16(2.0 * xc16.astype(np.float64))
        r1_all[b, 5] = b1h
        r1_all[b, 6] = b1l
        r1_all[b, 7] = -ones
        r1_all[b, 8] = -ones

    # --- second-exp argument operands (batch independent) ---
    v = (np.stack([ci, cj]) - 112.0) / np.sqrt(2.0 * GAMMA)  # [2, NS]
    vh = _f16(v)
    vl = _f16(v - vh.astype(np.float64))
    vs = vh.astype(np.float64) + vl.astype(np.float64)
    f2 = (vs**2).sum(0)
    b2h, b2l = _hi_lo(np.log(W2) - f2)
    f2h, f2l = _hi_lo(f2)
    a2 = np.empty((K2, NS), np.float16)
    r2 = np.empty((K2, NS), np.float16)
    a2[0:2] = vh
    a2[2:4] = vh
    a2[4:6] = vl
    a2[6:8] = vl
    r2[0:2] = _f16(2.0 * vh.astype(np.float64))
    r2[2:4] = _f16(2.0 * vl.astype(np.float64))
    r2[4:6] = _f16(2.0 * vh.astype(np.float64))
    r2[6:8] = _f16(2.0 * vl.astype(np.float64))
    a2[8] = ones
    a2[9] = ones
    a2[10] = f2h
    a2[11] = f2l
    r2[8] = b2h
    r2[9] = b2l
    r2[10] = -ones
    r2[11] = -ones

    WHALF = 4 * NT * 128
    RHALF = 4 * CORE_COLS
    in_maps = []
    for k in range(NCORES):
        tiles = _tiles_for_core(k)
        wae = np.zeros((41, WHALF), np.float16)
        wao = np.zeros((41, WHALF), np.float16)
        rae = np.zeros((41, RHALF), np.float16)
        rao = np.zeros((41, RHALF), np.float16)
        gek = np.zeros((K2, GE_COLS), np.float16)
        for t, (X, j, n) in enumerate(tiles):
            rows = slice(128 * X, 128 * X + 128)
            cols = slice(128 * (X + j), 128 * (X + j) + 128 * n)
            o_t, w = OFFS[t], TW[t]
            gek[:, WE_COL[t] : WE_COL[t] + 128] = a2[:, rows]
            c = next(ci_ for ci_, ts in enumerate(CHUNK_TILES) if t in ts)
            toff = o_t - CHUNKS[c][0]
            gek[:, RE_BASE[c] + toff : RE_BASE[c] + toff + w] = r2[:, cols]
            for bi in range(4):
                for par, (w_d, r_d) in enumerate([(wae, rae), (wao, rao)]):
                    b = 2 * bi + par
                    sl = slice((bi * NT + t) * 128, (bi * NT + t + 1) * 128)
                    cl = slice(bi * CORE_COLS + o_t, bi * CORE_COLS + o_t + w)
                    w_d[0:KC, sl] = wc_all[b][:, rows]
                    w_d[32:41, sl] = a1_all[b][:, rows]
                    r_d[0:KC, cl] = c16[b][:, cols]
                    r_d[32:41, cl] = r1_all[b][:, cols]
        in_maps.append({"wae": wae, "wao": wao, "rae": rae, "rao": rao, "ge": gek})
    return in_maps


_NC_CACHE = {}


def _get_nc():
    if "nc" not in _NC_CACHE:
        _NC_CACHE["nc"] = build_nc()
    return _NC_CACHE["nc"]


def kernel(guidance, clusters, coords):
    guidance = np.asarray(guidance)
    clusters = np.asarray(clusters)
    coords = np.asarray(coords)
    in_maps = prepare_inputs(guidance, clusters, coords)
    nc = _get_nc()
    res = bass_utils.run_bass_kernel_spmd(nc, in_maps, list(range(NCORES)))
    # reassemble upper triangle, then mirror
    full = np.zeros((B, NS, NS), np.float32)
    for k in range(NCORES):
        o = res.results[k]["out"].astype(np.float32)  # [B, 128, CORE_COLS]
        for t, (X, j, n) in enumerate(_tiles_for_core(k)):
            rows = slice(128 * X, 128 * X + 128)
            cols = slice(128 * (X + j), 128 * (X + j) + 128 * n)
            full[:, rows, cols] = o[:, :, OFFS[t] : OFFS[t] + TW[t]]
    up = np.triu(full, 1)
    full = np.triu(full) + np.swapaxes(up, 1, 2)
    return full


# revision 15
# speedup vs baseline: 1.0477x; 1.0477x over previous
"""Trainium2 Bass kernel for nn_ContrastiveCRFLoss (self-contained).

Math: for each batch b and sample pair (n, m) over 2048 gathered pixels:
    out[b,n,m] = -(C[b,n,m] * (W1*exp(-cd - gd[b]/(2*BETA)) + W2*exp(-cd/(2*GAMMA))))
where C = cluster Gram, cd = squared coord distance, gd = squared guidance
distance.  The output is SYMMETRIC in (n, m): C is a Gram matrix and both
exp kernels depend on symmetric distances.

Device strategy (8 cores, upper-triangle only, mirrored on host):
  - The 2048x2048 pair grid is cut into 16 row-blocks of 128.  Block i needs
    only columns [128*i, 2048) (upper triangle incl. the diagonal block).
    Core k owns blocks k and 15-k: (16-k)+(k+1) = 17 col-units of 128 ->
    exactly 8 tiles of 256 cols + 1 tile of 128 cols per batch on EVERY
    core (SPMD-uniform shapes; per-core geometry lives in host staging).
  - Three fp16 matmul streams per tile, spread over all four PE row groups
    (tile_position 0/32/64/96) so independent streams run concurrently:
      even batches: Gram at rows 0,  exp1-arg at rows 32
      odd  batches: Gram at rows 64, exp1-arg at rows 96
      exp2-arg (batch-independent): rows 0 / 96 by chunk
    Exp arguments are produced directly in PSUM by augmenting the operands
    with ones/norm/log-weight rows (hi/lo fp16 splits keep this exact).
  - Inputs are packed so each parity loads with ONE [41 x cols] DMA (wc at
    group base, a1 at group base+32) - 6 input DMA triggers total.
  - Per 512-col chunk: ACT exp(PSUM)->fp16, add e1+e2 split GpSimd/DVE;
    DVE mult over 1024-col pC superchunks; one [128,2176] store per batch.
  - Host mirrors the strict upper triangle to reconstruct the full output.
"""

import numpy as np

import concourse.bass as bass
import concourse.mybir as mybir
import concourse.bass_utils as bass_utils
from concourse.tile import TileContext
from concourse.vector_clock import ScopedClock

F16 = mybir.dt.float16
F32 = mybir.dt.float32

# problem constants (hardcoded per the task contract)
ALPHA, BETA, GAMMA = 0.5, 0.15, 25.0
W1, W2, SHIFT = 10.0, 3.0, 0.0
B = 8
NS = 2048
NCORES = 8
KC, K1, K2 = 27, 9, 12
NT = 9                       # column tiles per batch per core
TW = [256] * 8 + [128]       # tile widths
# tile -> column offset in the 2176-wide per-batch strip.  Tiles 0-3 then
# tile 8 sit in the first 1152 cols (e2 group 0), tiles 4-7 in the last
# 1024 (e2 group 96).
OFFS = [0, 256, 512, 768, 1152, 1408, 1664, 1920, 1024]
CORE_COLS = 2176
CHUNKS = [(0, 512), (512, 512), (1024, 128), (1152, 512), (1664, 512)]
CHUNK_TILES = [(0, 1), (2, 3), (8,), (4, 5), (6, 7)]
G2E = [0, 0, 0, 96, 96]      # e2-phase row group by chunk
# pC/mult superchunks: (col offset, width, chunk indices)
SUPER = [(0, 1024, (0, 1)), (1024, 128, (2,)), (1152, 1024, (3, 4))]
# ge (e2 operand pack) column bookkeeping
WE_COL = {0: 0, 1: 128, 2: 256, 3: 384, 8: 512, 4: 1792, 5: 1920, 6: 2048, 7: 2176}
RE_BASE = {0: 640, 1: 1152, 2: 1664, 3: 2304, 4: 2816}
GE_COLS = 3328

# ---------------------------------------------------------------------------
# Walrus in this image rejects >1 sync wait per instruction. Split the Tile
# tail-drain's waits and any multi-wait instruction into single-wait NOPs.
# ---------------------------------------------------------------------------
_MAXW = 1


def _split_drain_and_barrier(self, tick_clock, wait_clock):
    probe = self.nc.sync.nop(nofuse=True)
    wait_clock.add_sem_waits(probe.ins, ScopedClock({None: tick_clock.global_clock}))
    si = probe.ins.sync_info
    waits = list(si.on_wait)
    probe.ins.sync_info = mybir.SyncInfo(
        on_wait=waits[:_MAXW], on_update=list(si.on_update)
    )
    for i in range(_MAXW, len(waits), _MAXW):
        n2 = self.nc.sync.nop(nofuse=True)
        n2.ins.sync_info = mybir.SyncInfo(on_wait=waits[i : i + _MAXW], on_update=[])
    self.nc.sync.drain()
    self.nc.all_engine_barrier()
    popped = self.nc._tile_sem_poison_stack.pop()
    assert popped is self._sem_poison
    self.nc.clear_and_free_semaphores(list(self.sems.allocated().values()))
    self.nc.all_engine_barrier()


def _split_multiwait_insts(nc):
    n_split = 0
    for fn in nc.m.functions:
        for bb in fn.blocks:
            insts = list(bb.instructions)
            new_insts = []
            changed = False
            for inst in insts:
                si = inst.sync_info
                waits = list(si.on_wait) if si is not None else []
                if len(waits) > _MAXW:
                    n_split += 1
                    changed = True
                    n_extra = len(waits) - _MAXW
                    for i in range(0, n_extra, _MAXW):
                        nop = mybir.InstNoOp(
                            name=nc.get_next_instruction_name(),
                            engine=inst.engine,
                            bass_nofuse=True,
                            sync_info=mybir.SyncInfo(
                                on_wait=waits[i : i + _MAXW], on_update=[]
                            ),
                        )
                        new_insts.append(nop)
                    inst.sync_info = mybir.SyncInfo(
                        on_wait=waits[n_extra:], on_update=list(si.on_update)
                    )
                new_insts.append(inst)
            if changed:
                bb.instructions = new_insts
    return n_split


def _install_tile_patch():
    TileContext._drain_and_barrier = _split_drain_and_barrier


def _tiles_for_core(k):
    """17 col-units as 8x256 + 1x128 tiles: (row_block, unit_start, n_units)."""
    WA, WB = 16 - k, k + 1
    A, Bb = k, 15 - k
    tiles = []
    for j in range(0, WA - 1, 2):
        tiles.append((A, j, 2))
    for j in range(0, WB - 1, 2):
        tiles.append((Bb, j, 2))
    if WA % 2 == 0:
        tiles.append((Bb, WB - 1, 1))
    else:
        tiles.append((A, WA - 1, 1))
    assert len(tiles) == NT and sum(n for _, _, n in tiles) == 17
    return tiles


# ---------------------------------------------------------------------------
# Device program (identical on all cores; data differs per core)
# ---------------------------------------------------------------------------

def build_nc():
    _install_tile_patch()
    nc = bass.Bass()
    WHALF = 4 * NT * 128   # 4608: per-parity lhsT slot columns
    RHALF = 4 * CORE_COLS  # 8704: per-parity rhs columns
    # 3-D with padded inner dim: keeps DMA descriptors at <=4.3KB (big
    # merged descriptors all land on one DMA engine at ~27 GB/s)
    wae = nc.declare_dram_parameter("wae", [41, 4, 1280], F16, isOutput=False)
    wao = nc.declare_dram_parameter("wao", [41, 4, 1280], F16, isOutput=False)
    rae = nc.declare_dram_parameter("rae", [41, 4, 2304], F16, isOutput=False)
    rao = nc.declare_dram_parameter("rao", [41, 4, 2304], F16, isOutput=False)
    ge = nc.declare_dram_parameter("ge", [K2, GE_COLS], F16, isOutput=False)
    out = nc.declare_dram_parameter("out", [B, 128, CORE_COLS], F16, isOutput=True)

    with TileContext(nc) as tc:
        with (
            tc.tile_pool(name="w", bufs=1) as wpool,
            tc.tile_pool(name="r", bufs=1) as rpool,
            tc.tile_pool(name="e2p", bufs=1) as e2pool,
            tc.tile_pool(name="sb", bufs=12) as sbpool,
            tc.tile_pool(name="sm", bufs=4) as smpool,
            tc.tile_pool(name="ob", bufs=3) as opool,
            tc.tile_pool(name="psA", bufs=4, space="PSUM") as psa,
            tc.tile_pool(name="psB", bufs=2, space="PSUM") as psb,
        ):
            GE = wpool.tile([128, GE_COLS], F16)
            W = wpool.tile([128, 2 * WHALF], F16)
            R = rpool.tile([128, 2 * RHALF], F16)
            # e2 operands first (e2 phase unblocks earliest)
            nc.sync.dma_start(GE[0:K2, 0:1792], ge[:, 0:1792])
            nc.sync.dma_start(GE[96 : 96 + K2, 1792:GE_COLS], ge[:, 1792:GE_COLS])
            # packed parity loads: wc at group base, a1 at base+32
            nc.sync.dma_start(R[0:41, 0:RHALF], rae[:, :, 0:2176])
            nc.sync.dma_start(W[0:41, 0:WHALF], wae[:, :, 0:1152])
            nc.sync.dma_start(R[64:105, RHALF : 2 * RHALF], rao[:, :, 0:2176])
            nc.sync.dma_start(W[64:105, WHALF : 2 * WHALF], wao[:, :, 0:1152])

            e2 = e2pool.tile([128, CORE_COLS], F16)

            # --- e2 phase: batch-independent second-exp kernel ---
            for c, (off, wd) in enumerate(CHUNKS):
                g = G2E[c]
                p2 = psa.tile([128, 512], F32, tag="pA", name=f"p2c{c}")
                for t in CHUNK_TILES[c]:
                    w = TW[t]
                    toff = OFFS[t] - off
                    nc.tensor.matmul(
                        p2[:, toff : toff + w],
                        GE[g : g + K2, WE_COL[t] : WE_COL[t] + 128],
                        GE[g : g + K2, RE_BASE[c] + toff : RE_BASE[c] + toff + w],
                        start=True,
                        stop=True,
                        tile_position=(g, 0),
                    )
                nc.scalar.activation(
                    e2[:, off : off + wd],
                    p2[:, 0:wd],
                    mybir.ActivationFunctionType.Exp,
                )

            # --- batch loop ---
            for b in range(B):
                par = b % 2
                gc = 0 if par == 0 else 64
                g1 = 32 if par == 0 else 96
                wbase = par * WHALF + (b // 2) * NT * 128
                rbase = par * RHALF + (b // 2) * CORE_COLS
                ob = opool.tile([128, CORE_COLS], F16, tag="ob")
                gps_chunks = {0, 1, 2} if par == 0 else {0, 1}
                e1s = {}
                for c, (off, wd) in enumerate(CHUNKS):
                    p1 = psa.tile([128, 512], F32, tag="pA", name=f"p1b{b}c{c}")
                    for t in CHUNK_TILES[c]:
                        w = TW[t]
                        toff = OFFS[t] - off
                        nc.tensor.matmul(
                            p1[:, toff : toff + w],
                            W[g1 : g1 + K1, wbase + t * 128 : wbase + (t + 1) * 128],
                            R[g1 : g1 + K1, rbase + OFFS[t] : rbase + OFFS[t] + w],
                            start=True,
                            stop=True,
                            tile_position=(g1, 0),
                        )
                    e1 = sbpool.tile([128, 512], F16, tag="e1")
                    nc.scalar.activation(
                        e1[:, 0:wd],
                        p1[:, 0:wd],
                        mybir.ActivationFunctionType.Exp,
                    )
                    e1s[c] = e1
                for so, swd, chs in SUPER:
                    pC = psb.tile([128, 1024], F32, tag="pB", name=f"pCb{b}o{so}")
                    s = smpool.tile([128, 1024], F16, tag="s")
                    for c in chs:
                        off, wd = CHUNKS[c]
                        for t in CHUNK_TILES[c]:
                            w = TW[t]
                            toff = OFFS[t] - so
                            nc.tensor.matmul(
                                pC[:, toff : toff + w],
                                W[gc : gc + KC, wbase + t * 128 : wbase + (t + 1) * 128],
                                R[gc : gc + KC, rbase + OFFS[t] : rbase + OFFS[t] + w],
                                start=True,
                                stop=True,
                                tile_position=(gc, 0),
                            )
                        soff = off - so
                        addfn = (
                            nc.gpsimd.tensor_add
                            if c in gps_chunks
                            else nc.vector.tensor_add
                        )
                        addfn(
                            s[:, soff : soff + wd],
                            e1s[c][:, 0:wd],
                            e2[:, off : off + wd],
                        )
                    nc.vector.tensor_tensor(
                        ob[:, so : so + swd],
                        pC[:, 0:swd],
                        s[:, 0:swd],
                        mybir.AluOpType.mult,
                    )
                nc.sync.dma_start(out[b], ob[:])

    _split_multiwait_insts(nc)
    return nc


# ---------------------------------------------------------------------------
# Host-side input prep
# ---------------------------------------------------------------------------

def _f16(x):
    return np.asarray(x, dtype=np.float16)


def _hi_lo(x):
    """Split fp64 vector into two fp16 rows summing to ~x."""
    hi = _f16(x)
    lo = _f16(x - hi.astype(np.float64))
    return hi, lo


def prepare_inputs(guidance, clusters, coords):
    ci = np.asarray(coords[0], dtype=np.int64)
    cj = np.asarray(coords[1], dtype=np.int64)
    sel_g = guidance[:, :, ci, cj].astype(np.float64)  # [B, 3, NS]
    sel_c = clusters[:, :, ci, cj].astype(np.float32)  # [B, 27, NS]

    # --- cluster Gram operands (fp16 snap) ---
    c16 = _f16(sel_c)
    wc_all = -c16  # lhsT (negated -> folds the leading minus)

    # --- first-exp argument operands: arg1 = -cd - gd/(2*beta) + ln(W1) ---
    u16 = _f16(sel_g / np.sqrt(2.0 * BETA))  # [B, 3, NS]
    xc16 = _f16(np.stack([ci, cj]) - 112.0)  # [2, NS] exact
    f1 = (u16.astype(np.float64) ** 2).sum(1) + (
        xc16.astype(np.float64) ** 2
    ).sum(0)  # [B, NS]
    a1_all = np.empty((B, K1, NS), np.float16)
    r1_all = np.empty((B, K1, NS), np.float16)
    ones = np.ones(NS, np.float16)
    for b in range(B):
        b1h, b1l = _hi_lo(np.log(W1) - f1[b])
        f1h, f1l = _hi_lo(f1[b])
        a1_all[b, 0:3] = u16[b]
        a1_all[b, 3:5] = xc16
        a1_all[b, 5] = ones
        a1_all[b, 6] = ones
        a1_all[b, 7] = f1h
        a1_all[b, 8] = f1l
        r1_all[b, 0:3] = _f16(2.0 * u16[b].astype(np.float64))
        r1_all[b, 3:5] = _f16(2.0 * xc16.astype(np.float64))
        r1_all[b, 5] = b1h
        r1_all[b, 6] = b1l
        r1_all[b, 7] = -ones
        r1_all[b, 8] = -ones

    # --- second-exp argument operands (batch independent) ---
    v = (np.stack([ci, cj]) - 112.0) / np.sqrt(2.0 * GAMMA)  # [2, NS]
    vh = _f16(v)
    vl = _f16(v - vh.astype(np.float64))
    vs = vh.astype(np.float64) + vl.astype(np.float64)
    f2 = (vs**2).sum(0)
    b2h, b2l = _hi_lo(np.log(W2) - f2)
    f2h, f2l = _hi_lo(f2)
    a2 = np.empty((K2, NS), np.float16)
    r2 = np.empty((K2, NS), np.float16)
    a2[0:2] = vh
    a2[2:4] = vh
    a2[4:6] = vl
    a2[6:8] = vl
    r2[0:2] = _f16(2.0 * vh.astype(np.float64))
    r2[2:4] = _f16(2.0 * vl.astype(np.float64))
    r2[4:6] = _f16(2.0 * vh.astype(np.float64))
    r2[6:8] = _f16(2.0 * vl.astype(np.float64))
    a2[8] = ones
    a2[9] = ones
    a2[10] = f2h
    a2[11] = f2l
    r2[8] = b2h
    r2[9] = b2l
    r2[10] = -ones
    r2[11] = -ones

    WHALF = 4 * NT * 128
    RHALF = 4 * CORE_COLS
    in_maps = []
    for k in range(NCORES):
        tiles = _tiles_for_core(k)
        wae = np.zeros((41, WHALF), np.float16)
        wao = np.zeros((41, WHALF), np.float16)
        rae = np.zeros((41, RHALF), np.float16)
        rao = np.zeros((41, RHALF), np.float16)
        gek = np.zeros((K2, GE_COLS), np.float16)
        for t, (X, j, n) in enumerate(tiles):
            rows = slice(128 * X, 128 * X + 128)
            cols = slice(128 * (X + j), 128 * (X + j) + 128 * n)
            o_t, w = OFFS[t], TW[t]
            gek[:, WE_COL[t] : WE_COL[t] + 128] = a2[:, rows]
            c = next(ci_ for ci_, ts in enumerate(CHUNK_TILES) if t in ts)
            toff = o_t - CHUNKS[c][0]
            gek[:, RE_BASE[c] + toff : RE_BASE[c] + toff + w] = r2[:, cols]
            for bi in range(4):
                for par, (w_d, r_d) in enumerate([(wae, rae), (wao, rao)]):
                    b = 2 * bi + par
                    sl = slice((bi * NT + t) * 128, (bi * NT + t + 1) * 128)
                    cl = slice(bi * CORE_COLS + o_t, bi * CORE_COLS + o_t + w)
                    w_d[0:KC, sl] = wc_all[b][:, rows]
                    w_d[32:41, sl] = a1_all[b][:, rows]
                    r_d[0:KC, cl] = c16[b][:, cols]
                    r_d[32:41, cl] = r1_all[b][:, cols]
        def _pad3(a, nblk, blkw, padw):
            out_ = np.zeros((a.shape[0], nblk, padw), a.dtype)
            out_[:, :, :blkw] = a.reshape(a.shape[0], nblk, blkw)
            return out_

        in_maps.append(
            {
                "wae": _pad3(wae, 4, 1152, 1280),
                "wao": _pad3(wao, 4, 1152, 1280),
                "rae": _pad3(rae, 4, 2176, 2304),
                "rao": _pad3(rao, 4, 2176, 2304),
                "ge": gek,
            }
        )
    return in_maps


_NC_CACHE = {}


def _get_nc():
    if "nc" not in _NC_CACHE:
        _NC_CACHE["nc"] = build_nc()
    return _NC_CACHE["nc"]


def kernel(guidance, clusters, coords):
    guidance = np.asarray(guidance)
    clusters = np.asarray(clusters)
    coords = np.asarray(coords)
    in_maps = prepare_inputs(guidance, clusters, coords)
    nc = _get_nc()
    res = bass_utils.run_bass_kernel_spmd(nc, in_maps, list(range(NCORES)))
    # reassemble upper triangle, then mirror
    full = np.zeros((B, NS, NS), np.float32)
    for k in range(NCORES):
        o = res.results[k]["out"].astype(np.float32)  # [B, 128, CORE_COLS]
        for t, (X, j, n) in enumerate(_tiles_for_core(k)):
            rows = slice(128 * X, 128 * X + 128)
            cols = slice(128 * (X + j), 128 * (X + j) + 128 * n)
            full[:, rows, cols] = o[:, :, OFFS[t] : OFFS[t] + TW[t]]
    up = np.triu(full, 1)
    full = np.triu(full) + np.swapaxes(up, 1, 2)
    return full


# revision 18
# speedup vs baseline: 1.0677x; 1.0190x over previous
"""Trainium2 Bass kernel for nn_ContrastiveCRFLoss (self-contained).

Math: for each batch b and sample pair (n, m) over 2048 gathered pixels:
    out[b,n,m] = -(C[b,n,m] * (W1*exp(-cd - gd[b]/(2*BETA)) + W2*exp(-cd/(2*GAMMA))))
where C = cluster Gram, cd = squared coord distance, gd = squared guidance
distance.  The output is SYMMETRIC in (n, m): C is a Gram matrix and both
exp kernels depend on symmetric distances.

Device strategy (8 cores, upper-triangle only, mirrored on host):
  - The 2048x2048 pair grid is cut into 16 row-blocks of 128.  Block i needs
    only columns [128*i, 2048) (upper triangle incl. the diagonal block).
    Core k owns blocks k and 15-k: (16-k)+(k+1) = 17 col-units of 128 ->
    exactly 8 tiles of 256 cols + 1 tile of 128 cols per batch on EVERY
    core (SPMD-uniform shapes; per-core geometry lives in host staging).
  - Three fp16 matmul streams per tile, spread over all four PE row groups
    (tile_position 0/32/64/96) so independent streams run concurrently:
      even batches: Gram at rows 0,  exp1-arg at rows 32
      odd  batches: Gram at rows 64, exp1-arg at rows 96
      exp2-arg (batch-independent): rows 0 / 96 by chunk
    Exp arguments are produced directly in PSUM by augmenting the operands
    with ones/norm/log-weight rows (hi/lo fp16 splits keep this exact).
  - Inputs are packed so each parity loads with ONE [41 x cols] DMA (wc at
    group base, a1 at group base+32) - 6 input DMA triggers total.
  - Per 512-col chunk: ACT exp(PSUM)->fp16, add e1+e2 split GpSimd/DVE;
    DVE mult over 1024-col pC superchunks; one [128,2176] store per batch.
  - Host mirrors the strict upper triangle to reconstruct the full output.
"""

import numpy as np

import concourse.bass as bass
import concourse.mybir as mybir
import concourse.bass_utils as bass_utils
from concourse.tile import TileContext
from concourse.vector_clock import ScopedClock

F16 = mybir.dt.float16
F32 = mybir.dt.float32

# problem constants (hardcoded per the task contract)
ALPHA, BETA, GAMMA = 0.5, 0.15, 25.0
W1, W2, SHIFT = 10.0, 3.0, 0.0
B = 8
NS = 2048
NCORES = 8
KC, K1, K2 = 27, 9, 12
NT = 9                       # column tiles per batch per core
TW = [256] * 8 + [128]       # tile widths
# tile -> column offset in the 2176-wide per-batch strip.  Tiles 0-3 then
# tile 8 sit in the first 1152 cols (e2 group 0), tiles 4-7 in the last
# 1024 (e2 group 96).
OFFS = [0, 256, 512, 768, 1152, 1408, 1664, 1920, 1024]
CORE_COLS = 2176
CHUNKS = [(0, 512), (512, 512), (1024, 128), (1152, 512), (1664, 512)]
CHUNK_TILES = [(0, 1), (2, 3), (8,), (4, 5), (6, 7)]
G2E = [0, 0, 0, 96, 96]      # e2-phase row group by chunk
# pC/mult superchunks: (col offset, width, chunk indices)
SUPER = [(0, 1024, (0, 1)), (1024, 128, (2,)), (1152, 1024, (3, 4))]
# ge (e2 operand pack) column bookkeeping
WE_COL = {0: 0, 1: 128, 2: 256, 3: 384, 8: 512, 4: 1792, 5: 1920, 6: 2048, 7: 2176}
RE_BASE = {0: 640, 1: 1152, 2: 1664, 3: 2304, 4: 2816}
GE_COLS = 3328

# ---------------------------------------------------------------------------
# Walrus in this image rejects >1 sync wait per instruction. Split the Tile
# tail-drain's waits and any multi-wait instruction into single-wait NOPs.
# ---------------------------------------------------------------------------
_MAXW = 1


def _split_drain_and_barrier(self, tick_clock, wait_clock):
    probe = self.nc.sync.nop(nofuse=True)
    wait_clock.add_sem_waits(probe.ins, ScopedClock({None: tick_clock.global_clock}))
    si = probe.ins.sync_info
    waits = list(si.on_wait)
    probe.ins.sync_info = mybir.SyncInfo(
        on_wait=waits[:_MAXW], on_update=list(si.on_update)
    )
    for i in range(_MAXW, len(waits), _MAXW):
        n2 = self.nc.sync.nop(nofuse=True)
        n2.ins.sync_info = mybir.SyncInfo(on_wait=waits[i : i + _MAXW], on_update=[])
    self.nc.sync.drain()
    self.nc.all_engine_barrier()
    popped = self.nc._tile_sem_poison_stack.pop()
    assert popped is self._sem_poison
    self.nc.clear_and_free_semaphores(list(self.sems.allocated().values()))
    self.nc.all_engine_barrier()


def _split_multiwait_insts(nc):
    n_split = 0
    for fn in nc.m.functions:
        for bb in fn.blocks:
            insts = list(bb.instructions)
            new_insts = []
            changed = False
            for inst in insts:
                si = inst.sync_info
                waits = list(si.on_wait) if si is not None else []
                if len(waits) > _MAXW:
                    n_split += 1
                    changed = True
                    n_extra = len(waits) - _MAXW
                    for i in range(0, n_extra, _MAXW):
                        nop = mybir.InstNoOp(
                            name=nc.get_next_instruction_name(),
                            engine=inst.engine,
                            bass_nofuse=True,
                            sync_info=mybir.SyncInfo(
                                on_wait=waits[i : i + _MAXW], on_update=[]
                            ),
                        )
                        new_insts.append(nop)
                    inst.sync_info = mybir.SyncInfo(
                        on_wait=waits[n_extra:], on_update=list(si.on_update)
                    )
                new_insts.append(inst)
            if changed:
                bb.instructions = new_insts
    return n_split


def _install_tile_patch():
    TileContext._drain_and_barrier = _split_drain_and_barrier


def _tiles_for_core(k):
    """17 col-units as 8x256 + 1x128 tiles: (row_block, unit_start, n_units)."""
    WA, WB = 16 - k, k + 1
    A, Bb = k, 15 - k
    tiles = []
    for j in range(0, WA - 1, 2):
        tiles.append((A, j, 2))
    for j in range(0, WB - 1, 2):
        tiles.append((Bb, j, 2))
    if WA % 2 == 0:
        tiles.append((Bb, WB - 1, 1))
    else:
        tiles.append((A, WA - 1, 1))
    assert len(tiles) == NT and sum(n for _, _, n in tiles) == 17
    return tiles


# ---------------------------------------------------------------------------
# Device program (identical on all cores; data differs per core)
# ---------------------------------------------------------------------------

def build_nc():
    _install_tile_patch()
    nc = bass.Bass()
    WHALF = 4 * NT * 128   # 4608: per-parity lhsT slot columns
    RHALF = 4 * CORE_COLS  # 8704: per-parity rhs columns
    # flat 2-D: one ~17KB descriptor per partition (41/load) keeps the
    # total outstanding descriptor count under the DGE ring limit --
    # overflowing the ring degrades to a serial single-engine drain
    wae = nc.declare_dram_parameter("wae", [41, WHALF], F16, isOutput=False)
    wao = nc.declare_dram_parameter("wao", [41, WHALF], F16, isOutput=False)
    rae = nc.declare_dram_parameter("rae", [41, RHALF], F16, isOutput=False)
    rao = nc.declare_dram_parameter("rao", [41, RHALF], F16, isOutput=False)
    ge = nc.declare_dram_parameter("ge", [K2, GE_COLS], F16, isOutput=False)
    out = nc.declare_dram_parameter("out", [B, 128, CORE_COLS], F16, isOutput=True)

    with TileContext(nc) as tc:
        with (
            tc.tile_pool(name="w", bufs=1) as wpool,
            tc.tile_pool(name="r", bufs=1) as rpool,
            tc.tile_pool(name="e2p", bufs=1) as e2pool,
            tc.tile_pool(name="sb", bufs=12) as sbpool,
            tc.tile_pool(name="sm", bufs=4) as smpool,
            tc.tile_pool(name="ob", bufs=3) as opool,
            tc.tile_pool(name="psA", bufs=4, space="PSUM") as psa,
            tc.tile_pool(name="psB", bufs=2, space="PSUM") as psb,
        ):
            GE = wpool.tile([128, GE_COLS], F16)
            W = wpool.tile([128, 2 * WHALF], F16)
            R = rpool.tile([128, 2 * RHALF], F16)
            # e2 operands first (e2 phase unblocks earliest)
            nc.sync.dma_start(GE[0:K2, 0:1792], ge[:, 0:1792])
            nc.sync.dma_start(GE[96 : 96 + K2, 1792:GE_COLS], ge[:, 1792:GE_COLS])
            # packed parity loads: wc at group base, a1 at base+32
            nc.sync.dma_start(R[0:41, 0:RHALF], rae[:])
            nc.sync.dma_start(W[0:41, 0:WHALF], wae[:])
            nc.sync.dma_start(R[64:105, RHALF : 2 * RHALF], rao[:])
            nc.sync.dma_start(W[64:105, WHALF : 2 * WHALF], wao[:])

            e2 = e2pool.tile([128, CORE_COLS], F16)

            # --- e2 phase: batch-independent second-exp kernel ---
            for c, (off, wd) in enumerate(CHUNKS):
                g = G2E[c]
                p2 = psa.tile([128, 512], F32, tag="pA", name=f"p2c{c}")
                for t in CHUNK_TILES[c]:
                    w = TW[t]
                    toff = OFFS[t] - off
                    nc.tensor.matmul(
                        p2[:, toff : toff + w],
                        GE[g : g + K2, WE_COL[t] : WE_COL[t] + 128],
                        GE[g : g + K2, RE_BASE[c] + toff : RE_BASE[c] + toff + w],
                        start=True,
                        stop=True,
                        tile_position=(g, 0),
                    )
                nc.scalar.activation(
                    e2[:, off : off + wd],
                    p2[:, 0:wd],
                    mybir.ActivationFunctionType.Exp,
                )

            # --- batch loop ---
            for b in range(B):
                par = b % 2
                gc = 0 if par == 0 else 64
                g1 = 32 if par == 0 else 96
                wbase = par * WHALF + (b // 2) * NT * 128
                rbase = par * RHALF + (b // 2) * CORE_COLS
                ob = opool.tile([128, CORE_COLS], F16, tag="ob")
                gps_chunks = {0, 1, 2} if par == 0 else {0, 1}
                e1s = {}
                for c, (off, wd) in enumerate(CHUNKS):
                    p1 = psa.tile([128, 512], F32, tag="pA", name=f"p1b{b}c{c}")
                    for t in CHUNK_TILES[c]:
                        w = TW[t]
                        toff = OFFS[t] - off
                        nc.tensor.matmul(
                            p1[:, toff : toff + w],
                            W[g1 : g1 + K1, wbase + t * 128 : wbase + (t + 1) * 128],
                            R[g1 : g1 + K1, rbase + OFFS[t] : rbase + OFFS[t] + w],
                            start=True,
                            stop=True,
                            tile_position=(g1, 0),
                        )
                    e1 = sbpool.tile([128, 512], F16, tag="e1")
                    nc.scalar.activation(
                        e1[:, 0:wd],
                        p1[:, 0:wd],
                        mybir.ActivationFunctionType.Exp,
                    )
                    e1s[c] = e1
                for so, swd, chs in SUPER:
                    pC = psb.tile([128, 1024], F32, tag="pB", name=f"pCb{b}o{so}")
                    s = smpool.tile([128, 1024], F16, tag="s")
                    for c in chs:
                        off, wd = CHUNKS[c]
                        for t in CHUNK_TILES[c]:
                            w = TW[t]
                            toff = OFFS[t] - so
                            nc.tensor.matmul(
                                pC[:, toff : toff + w],
                                W[gc : gc + KC, wbase + t * 128 : wbase + (t + 1) * 128],
                                R[gc : gc + KC, rbase + OFFS[t] : rbase + OFFS[t] + w],
                                start=True,
                                stop=True,
                                tile_position=(gc, 0),
                            )
                        soff = off - so
                        addfn = (
                            nc.gpsimd.tensor_add
                            if c in gps_chunks
                            else nc.vector.tensor_add
                        )
                        addfn(
                            s[:, soff : soff + wd],
                            e1s[c][:, 0:wd],
                            e2[:, off : off + wd],
                        )
                    nc.vector.tensor_tensor(
                        ob[:, so : so + swd],
                        pC[:, 0:swd],
                        s[:, 0:swd],
                        mybir.AluOpType.mult,
                    )
                nc.sync.dma_start(out[b], ob[:])

    _split_multiwait_insts(nc)
    return nc


# ---------------------------------------------------------------------------
# Host-side input prep
# ---------------------------------------------------------------------------

def _f16(x):
    return np.asarray(x, dtype=np.float16)


def _hi_lo(x):
    """Split fp64 vector into two fp16 rows summing to ~x."""
    hi = _f16(x)
    lo = _f16(x - hi.astype(np.float64))
    return hi, lo


def prepare_inputs(guidance, clusters, coords):
    ci = np.asarray(coords[0], dtype=np.int64)
    cj = np.asarray(coords[1], dtype=np.int64)
    sel_g = guidance[:, :, ci, cj].astype(np.float64)  # [B, 3, NS]
    sel_c = clusters[:, :, ci, cj].astype(np.float32)  # [B, 27, NS]

    # --- cluster Gram operands (fp16 snap) ---
    c16 = _f16(sel_c)
    wc_all = -c16  # lhsT (negated -> folds the leading minus)

    # --- first-exp argument operands: arg1 = -cd - gd/(2*beta) + ln(W1) ---
    u16 = _f16(sel_g / np.sqrt(2.0 * BETA))  # [B, 3, NS]
    xc16 = _f16(np.stack([ci, cj]) - 112.0)  # [2, NS] exact
    f1 = (u16.astype(np.float64) ** 2).sum(1) + (
        xc16.astype(np.float64) ** 2
    ).sum(0)  # [B, NS]
    a1_all = np.empty((B, K1, NS), np.float16)
    r1_all = np.empty((B, K1, NS), np.float16)
    ones = np.ones(NS, np.float16)
    for b in range(B):
        b1h, b1l = _hi_lo(np.log(W1) - f1[b])
        f1h, f1l = _hi_lo(f1[b])
        a1_all[b, 0:3] = u16[b]
        a1_all[b, 3:5] = xc16
        a1_all[b, 5] = ones
        a1_all[b, 6] = ones
        a1_all[b, 7] = f1h
        a1_all[b, 8] = f1l
        r1_all[b, 0:3] = _f16(2.0 * u16[b].astype(np.float64))
        r1_all[b, 3:5] = _f16(2.0 * xc16.astype(np.float64))
        r1_all[b, 5] = b1h
        r1_all[b, 6] = b1l
        r1_all[b, 7] = -ones
        r1_all[b, 8] = -ones

    # --- second-exp argument operands (batch independent) ---
    v = (np.stack([ci, cj]) - 112.0) / np.sqrt(2.0 * GAMMA)  # [2, NS]
    vh = _f16(v)
    vl = _f16(v - vh.astype(np.float64))
    vs = vh.astype(np.float64) + vl.astype(np.float64)
    f2 = (vs**2).sum(0)
    b2h, b2l = _hi_lo(np.log(W2) - f2)
    f2h, f2l = _hi_lo(f2)
    a2 = np.empty((K2, NS), np.float16)
    r2 = np.empty((K2, NS), np.float16)
    a2[0:2] = vh
    a2[2:4] = vh
    a2[4:6] = vl
    a2[6:8] = vl
    r2[0:2] = _f16(2.0 * vh.astype(np.float64))
    r2[2:4] = _f16(2.0 * vl.astype(np.float64))
    r2[4:6] = _f16(2.0 * vh.astype(np.float64))
    r2[6:8] = _f16(2.0 * vl.astype(np.float64))
    a2[8] = ones
    a2[9] = ones
    a2[10] = f2h
    a2[11] = f2l
    r2[8] = b2h
    r2[9] = b2l
    r2[10] = -ones
    r2[11] = -ones

    WHALF = 4 * NT * 128
    RHALF = 4 * CORE_COLS
    in_maps = []
    for k in range(NCORES):
        tiles = _tiles_for_core(k)
        wae = np.zeros((41, WHALF), np.float16)
        wao = np.zeros((41, WHALF), np.float16)
        rae = np.zeros((41, RHALF), np.float16)
        rao = np.zeros((41, RHALF), np.float16)
        gek = np.zeros((K2, GE_COLS), np.float16)
        for t, (X, j, n) in enumerate(tiles):
            rows = slice(128 * X, 128 * X + 128)
            cols = slice(128 * (X + j), 128 * (X + j) + 128 * n)
            o_t, w = OFFS[t], TW[t]
            gek[:, WE_COL[t] : WE_COL[t] + 128] = a2[:, rows]
            c = next(ci_ for ci_, ts in enumerate(CHUNK_TILES) if t in ts)
            toff = o_t - CHUNKS[c][0]
            gek[:, RE_BASE[c] + toff : RE_BASE[c] + toff + w] = r2[:, cols]
            for bi in range(4):
                for par, (w_d, r_d) in enumerate([(wae, rae), (wao, rao)]):
                    b = 2 * bi + par
                    sl = slice((bi * NT + t) * 128, (bi * NT + t + 1) * 128)
                    cl = slice(bi * CORE_COLS + o_t, bi * CORE_COLS + o_t + w)
                    w_d[0:KC, sl] = wc_all[b][:, rows]
                    w_d[32:41, sl] = a1_all[b][:, rows]
                    r_d[0:KC, cl] = c16[b][:, cols]
                    r_d[32:41, cl] = r1_all[b][:, cols]
        in_maps.append({"wae": wae, "wao": wao, "rae": rae, "rao": rao, "ge": gek})
    return in_maps


_NC_CACHE = {}


def _get_nc():
    if "nc" not in _NC_CACHE:
        _NC_CACHE["nc"] = build_nc()
    return _NC_CACHE["nc"]


def kernel(guidance, clusters, coords):
    guidance = np.asarray(guidance)
    clusters = np.asarray(clusters)
    coords = np.asarray(coords)
    in_maps = prepare_inputs(guidance, clusters, coords)
    nc = _get_nc()
    res = bass_utils.run_bass_kernel_spmd(nc, in_maps, list(range(NCORES)))
    # reassemble upper triangle, then mirror
    full = np.zeros((B, NS, NS), np.float32)
    for k in range(NCORES):
        o = res.results[k]["out"].astype(np.float32)  # [B, 128, CORE_COLS]
        for t, (X, j, n) in enumerate(_tiles_for_core(k)):
            rows = slice(128 * X, 128 * X + 128)
            cols = slice(128 * (X + j), 128 * (X + j) + 128 * n)
            full[:, rows, cols] = o[:, :, OFFS[t] : OFFS[t] + TW[t]]
    up = np.triu(full, 1)
    full = np.triu(full) + np.swapaxes(up, 1, 2)
    return full


# revision 22
# speedup vs baseline: 2.2543x; 2.1114x over previous
"""Trainium2 Bass kernel for nn_ContrastiveCRFLoss (self-contained).

Math: for each batch b and sample pair (n, m) over 2048 gathered pixels:
    out[b,n,m] = -(C[b,n,m] * (W1*exp(-cd - gd[b]/(2*BETA)) + W2*exp(-cd/(2*GAMMA))))
where C = cluster Gram, cd = squared coord distance, gd = squared guidance
distance.  The output is SYMMETRIC in (n, m): C is a Gram matrix and both
exp kernels depend on symmetric distances.

Device strategy (8 cores, upper-triangle only, mirrored on host):
  - The 2048x2048 pair grid is cut into 16 row-blocks of 128.  Block i needs
    only columns [128*i, 2048) (upper triangle incl. the diagonal block).
    Core k owns blocks k and 15-k: (16-k)+(k+1) = 17 col-units of 128 ->
    exactly 8 tiles of 256 cols + 1 tile of 128 cols per batch on EVERY
    core (SPMD-uniform shapes; per-core geometry lives in host staging).
  - Three fp16 matmul streams per tile, spread over all four PE row groups
    (tile_position 0/32/64/96) so independent streams run concurrently:
      even batches: Gram at rows 0,  exp1-arg at rows 32
      odd  batches: Gram at rows 64, exp1-arg at rows 96
      exp2-arg (batch-independent): rows 0 / 96 by chunk
    Exp arguments are produced directly in PSUM by augmenting the operands
    with ones/norm/log-weight rows (hi/lo fp16 splits keep this exact).
  - Inputs are packed so each parity loads with ONE [41 x cols] DMA (wc at
    group base, a1 at group base+32) - 6 input DMA triggers total.
  - Per 512-col chunk: ACT exp(PSUM)->fp16, add e1+e2 split GpSimd/DVE;
    DVE mult over 1024-col pC superchunks; one [128,2176] store per batch.
  - Host mirrors the strict upper triangle to reconstruct the full output.
"""

import numpy as np

import concourse.bass as bass
import concourse.mybir as mybir
import concourse.bass_utils as bass_utils
from concourse.tile import TileContext
from concourse.vector_clock import ScopedClock

F16 = mybir.dt.float16
F32 = mybir.dt.float32

# problem constants (hardcoded per the task contract)
ALPHA, BETA, GAMMA = 0.5, 0.15, 25.0
W1, W2, SHIFT = 10.0, 3.0, 0.0
B = 8
NS = 2048
NCORES = 8
KC, K1, K2 = 27, 9, 12
NT = 9                       # column tiles per batch per core
TW = [256] * 8 + [128]       # tile widths
# tile -> column offset in the 2176-wide per-batch strip.  Tiles 0-3 then
# tile 8 sit in the first 1152 cols (e2 group 0), tiles 4-7 in the last
# 1024 (e2 group 96).
OFFS = [0, 256, 512, 768, 1152, 1408, 1664, 1920, 1024]
CORE_COLS = 2176
CHUNKS = [(0, 512), (512, 512), (1024, 128), (1152, 512), (1664, 512)]
CHUNK_TILES = [(0, 1), (2, 3), (8,), (4, 5), (6, 7)]
G2E = [0, 0, 0, 96, 96]      # e2-phase row group by chunk
# pC/mult superchunks: (col offset, width, chunk indices)
SUPER = [(0, 1024, (0, 1)), (1024, 128, (2,)), (1152, 1024, (3, 4))]
# ge (e2 operand pack) column bookkeeping
WE_COL = {0: 0, 1: 128, 2: 256, 3: 384, 8: 512, 4: 1792, 5: 1920, 6: 2048, 7: 2176}
RE_BASE = {0: 640, 1: 1152, 2: 1664, 3: 2304, 4: 2816}
GE_COLS = 3328

# ---------------------------------------------------------------------------
# Walrus in this image rejects >1 sync wait per instruction. Split the Tile
# tail-drain's waits and any multi-wait instruction into single-wait NOPs.
# ---------------------------------------------------------------------------
_MAXW = 1


def _split_drain_and_barrier(self, tick_clock, wait_clock):
    probe = self.nc.sync.nop(nofuse=True)
    wait_clock.add_sem_waits(probe.ins, ScopedClock({None: tick_clock.global_clock}))
    si = probe.ins.sync_info
    waits = list(si.on_wait)
    probe.ins.sync_info = mybir.SyncInfo(
        on_wait=waits[:_MAXW], on_update=list(si.on_update)
    )
    for i in range(_MAXW, len(waits), _MAXW):
        n2 = self.nc.sync.nop(nofuse=True)
        n2.ins.sync_info = mybir.SyncInfo(on_wait=waits[i : i + _MAXW], on_update=[])
    self.nc.sync.drain()
    self.nc.all_engine_barrier()
    popped = self.nc._tile_sem_poison_stack.pop()
    assert popped is self._sem_poison
    self.nc.clear_and_free_semaphores(list(self.sems.allocated().values()))
    self.nc.all_engine_barrier()


def _split_multiwait_insts(nc):
    n_split = 0
    for fn in nc.m.functions:
        for bb in fn.blocks:
            insts = list(bb.instructions)
            new_insts = []
            changed = False
            for inst in insts:
                si = inst.sync_info
                waits = list(si.on_wait) if si is not None else []
                if len(waits) > _MAXW:
                    n_split += 1
                    changed = True
                    n_extra = len(waits) - _MAXW
                    for i in range(0, n_extra, _MAXW):
                        nop = mybir.InstNoOp(
                            name=nc.get_next_instruction_name(),
                            engine=inst.engine,
                            bass_nofuse=True,
                            sync_info=mybir.SyncInfo(
                                on_wait=waits[i : i + _MAXW], on_update=[]
                            ),
                        )
                        new_insts.append(nop)
                    inst.sync_info = mybir.SyncInfo(
                        on_wait=waits[n_extra:], on_update=list(si.on_update)
                    )
                new_insts.append(inst)
            if changed:
                bb.instructions = new_insts
    return n_split


def _install_tile_patch():
    TileContext._drain_and_barrier = _split_drain_and_barrier


def _tiles_for_core(k):
    """17 col-units as 8x256 + 1x128 tiles: (row_block, unit_start, n_units)."""
    WA, WB = 16 - k, k + 1
    A, Bb = k, 15 - k
    tiles = []
    for j in range(0, WA - 1, 2):
        tiles.append((A, j, 2))
    for j in range(0, WB - 1, 2):
        tiles.append((Bb, j, 2))
    if WA % 2 == 0:
        tiles.append((Bb, WB - 1, 1))
    else:
        tiles.append((A, WA - 1, 1))
    assert len(tiles) == NT and sum(n for _, _, n in tiles) == 17
    return tiles


# ---------------------------------------------------------------------------
# Device program (identical on all cores; data differs per core)
# ---------------------------------------------------------------------------

def build_nc():
    _install_tile_patch()
    nc = bass.Bass()
    WHALF = 4 * NT * 128   # 4608: per-parity lhsT slot columns
    RHALF = 4 * CORE_COLS  # 8704: per-parity rhs columns
    # Full-128-partition loads: DRAM->SBUF DMAs with few dest partitions
    # degrade to a single DMA engine (~27 GB/s); 128-partition transfers
    # stripe across all 16 engines like the stores do.  Even/odd parity
    # operands overlay the same columns at different partition groups.
    wf = nc.declare_dram_parameter("wf", [128, WHALF], F16, isOutput=False)
    rf = nc.declare_dram_parameter("rf", [128, RHALF], F16, isOutput=False)
    ge = nc.declare_dram_parameter("ge", [K2, GE_COLS], F16, isOutput=False)
    out = nc.declare_dram_parameter("out", [B, 128, CORE_COLS], F16, isOutput=True)

    with TileContext(nc) as tc:
        with (
            tc.tile_pool(name="w", bufs=1) as wpool,
            tc.tile_pool(name="r", bufs=1) as rpool,
            tc.tile_pool(name="e2p", bufs=1) as e2pool,
            tc.tile_pool(name="sb", bufs=12) as sbpool,
            tc.tile_pool(name="sm", bufs=4) as smpool,
            tc.tile_pool(name="ob", bufs=3) as opool,
            tc.tile_pool(name="psA", bufs=4, space="PSUM") as psa,
            tc.tile_pool(name="psB", bufs=2, space="PSUM") as psb,
        ):
            GE = wpool.tile([128, GE_COLS], F16)
            W = wpool.tile([128, WHALF], F16)
            R = rpool.tile([128, RHALF], F16)
            # e2 operands first (e2 phase unblocks earliest)
            nc.sync.dma_start(GE[0:K2, 0:1792], ge[:, 0:1792])
            nc.sync.dma_start(GE[96 : 96 + K2, 1792:GE_COLS], ge[:, 1792:GE_COLS])
            # full-partition loads in ~4KB/partition column chunks
            for c0, c1 in [(0, 2176), (2176, 4352), (4352, 6528), (6528, 8704)]:
                nc.sync.dma_start(R[:, c0:c1], rf[:, c0:c1])
            for c0, c1 in [(0, 2304), (2304, 4608)]:
                nc.sync.dma_start(W[:, c0:c1], wf[:, c0:c1])

            e2 = e2pool.tile([128, CORE_COLS], F16)

            # --- e2 phase: batch-independent second-exp kernel ---
            for c, (off, wd) in enumerate(CHUNKS):
                g = G2E[c]
                p2 = psa.tile([128, 512], F32, tag="pA", name=f"p2c{c}")
                for t in CHUNK_TILES[c]:
                    w = TW[t]
                    toff = OFFS[t] - off
                    nc.tensor.matmul(
                        p2[:, toff : toff + w],
                        GE[g : g + K2, WE_COL[t] : WE_COL[t] + 128],
                        GE[g : g + K2, RE_BASE[c] + toff : RE_BASE[c] + toff + w],
                        start=True,
                        stop=True,
                        tile_position=(g, 0),
                    )
                nc.scalar.activation(
                    e2[:, off : off + wd],
                    p2[:, 0:wd],
                    mybir.ActivationFunctionType.Exp,
                )

            # --- batch loop ---
            for b in range(B):
                par = b % 2
                gc = 0 if par == 0 else 64
                g1 = 32 if par == 0 else 96
                wbase = (b // 2) * NT * 128
                rbase = (b // 2) * CORE_COLS
                ob = opool.tile([128, CORE_COLS], F16, tag="ob")
                gps_chunks = {0, 1, 2} if par == 0 else {0, 1}
                e1s = {}
                for c, (off, wd) in enumerate(CHUNKS):
                    p1 = psa.tile([128, 512], F32, tag="pA", name=f"p1b{b}c{c}")
                    for t in CHUNK_TILES[c]:
                        w = TW[t]
                        toff = OFFS[t] - off
                        nc.tensor.matmul(
                            p1[:, toff : toff + w],
                            W[g1 : g1 + K1, wbase + t * 128 : wbase + (t + 1) * 128],
                            R[g1 : g1 + K1, rbase + OFFS[t] : rbase + OFFS[t] + w],
                            start=True,
                            stop=True,
                            tile_position=(g1, 0),
                        )
                    e1 = sbpool.tile([128, 512], F16, tag="e1")
                    nc.scalar.activation(
                        e1[:, 0:wd],
                        p1[:, 0:wd],
                        mybir.ActivationFunctionType.Exp,
                    )
                    e1s[c] = e1
                for so, swd, chs in SUPER:
                    pC = psb.tile([128, 1024], F32, tag="pB", name=f"pCb{b}o{so}")
                    s = smpool.tile([128, 1024], F16, tag="s")
                    for c in chs:
                        off, wd = CHUNKS[c]
                        for t in CHUNK_TILES[c]:
                            w = TW[t]
                            toff = OFFS[t] - so
                            nc.tensor.matmul(
                                pC[:, toff : toff + w],
                                W[gc : gc + KC, wbase + t * 128 : wbase + (t + 1) * 128],
                                R[gc : gc + KC, rbase + OFFS[t] : rbase + OFFS[t] + w],
                                start=True,
                                stop=True,
                                tile_position=(gc, 0),
                            )
                        soff = off - so
                        addfn = (
                            nc.gpsimd.tensor_add
                            if c in gps_chunks
                            else nc.vector.tensor_add
                        )
                        addfn(
                            s[:, soff : soff + wd],
                            e1s[c][:, 0:wd],
                            e2[:, off : off + wd],
                        )
                    nc.vector.tensor_tensor(
                        ob[:, so : so + swd],
                        pC[:, 0:swd],
                        s[:, 0:swd],
                        mybir.AluOpType.mult,
                    )
                nc.sync.dma_start(out[b], ob[:])

    _split_multiwait_insts(nc)
    return nc


# ---------------------------------------------------------------------------
# Host-side input prep
# ---------------------------------------------------------------------------

def _f16(x):
    return np.asarray(x, dtype=np.float16)


def _hi_lo(x):
    """Split fp64 vector into two fp16 rows summing to ~x."""
    hi = _f16(x)
    lo = _f16(x - hi.astype(np.float64))
    return hi, lo


def prepare_inputs(guidance, clusters, coords):
    ci = np.asarray(coords[0], dtype=np.int64)
    cj = np.asarray(coords[1], dtype=np.int64)
    sel_g = guidance[:, :, ci, cj].astype(np.float64)  # [B, 3, NS]
    sel_c = clusters[:, :, ci, cj].astype(np.float32)  # [B, 27, NS]

    # --- cluster Gram operands (fp16 snap) ---
    c16 = _f16(sel_c)
    wc_all = -c16  # lhsT (negated -> folds the leading minus)

    # --- first-exp argument operands: arg1 = -cd - gd/(2*beta) + ln(W1) ---
    u16 = _f16(sel_g / np.sqrt(2.0 * BETA))  # [B, 3, NS]
    xc16 = _f16(np.stack([ci, cj]) - 112.0)  # [2, NS] exact
    f1 = (u16.astype(np.float64) ** 2).sum(1) + (
        xc16.astype(np.float64) ** 2
    ).sum(0)  # [B, NS]
    a1_all = np.empty((B, K1, NS), np.float16)
    r1_all = np.empty((B, K1, NS), np.float16)
    ones = np.ones(NS, np.float16)
    for b in range(B):
        b1h, b1l = _hi_lo(np.log(W1) - f1[b])
        f1h, f1l = _hi_lo(f1[b])
        a1_all[b, 0:3] = u16[b]
        a1_all[b, 3:5] = xc16
        a1_all[b, 5] = ones
        a1_all[b, 6] = ones
        a1_all[b, 7] = f1h
        a1_all[b, 8] = f1l
        r1_all[b, 0:3] = _f16(2.0 * u16[b].astype(np.float64))
        r1_all[b, 3:5] = _f16(2.0 * xc16.astype(np.float64))
        r1_all[b, 5] = b1h
        r1_all[b, 6] = b1l
        r1_all[b, 7] = -ones
        r1_all[b, 8] = -ones

    # --- second-exp argument operands (batch independent) ---
    v = (np.stack([ci, cj]) - 112.0) / np.sqrt(2.0 * GAMMA)  # [2, NS]
    vh = _f16(v)
    vl = _f16(v - vh.astype(np.float64))
    vs = vh.astype(np.float64) + vl.astype(np.float64)
    f2 = (vs**2).sum(0)
    b2h, b2l = _hi_lo(np.log(W2) - f2)
    f2h, f2l = _hi_lo(f2)
    a2 = np.empty((K2, NS), np.float16)
    r2 = np.empty((K2, NS), np.float16)
    a2[0:2] = vh
    a2[2:4] = vh
    a2[4:6] = vl
    a2[6:8] = vl
    r2[0:2] = _f16(2.0 * vh.astype(np.float64))
    r2[2:4] = _f16(2.0 * vl.astype(np.float64))
    r2[4:6] = _f16(2.0 * vh.astype(np.float64))
    r2[6:8] = _f16(2.0 * vl.astype(np.float64))
    a2[8] = ones
    a2[9] = ones
    a2[10] = f2h
    a2[11] = f2l
    r2[8] = b2h
    r2[9] = b2l
    r2[10] = -ones
    r2[11] = -ones

    WHALF = 4 * NT * 128
    RHALF = 4 * CORE_COLS
    in_maps = []
    for k in range(NCORES):
        tiles = _tiles_for_core(k)
        wf = np.zeros((128, WHALF), np.float16)
        rf = np.zeros((128, RHALF), np.float16)
        gek = np.zeros((K2, GE_COLS), np.float16)
        for t, (X, j, n) in enumerate(tiles):
            rows = slice(128 * X, 128 * X + 128)
            cols = slice(128 * (X + j), 128 * (X + j) + 128 * n)
            o_t, w = OFFS[t], TW[t]
            gek[:, WE_COL[t] : WE_COL[t] + 128] = a2[:, rows]
            c = next(ci_ for ci_, ts in enumerate(CHUNK_TILES) if t in ts)
            toff = o_t - CHUNKS[c][0]
            gek[:, RE_BASE[c] + toff : RE_BASE[c] + toff + w] = r2[:, cols]
            for bi in range(4):
                for par, base in enumerate([0, 64]):
                    b = 2 * bi + par
                    sl = slice((bi * NT + t) * 128, (bi * NT + t + 1) * 128)
                    cl = slice(bi * CORE_COLS + o_t, bi * CORE_COLS + o_t + w)
                    wf[base : base + KC, sl] = wc_all[b][:, rows]
                    wf[base + 32 : base + 41, sl] = a1_all[b][:, rows]
                    rf[base : base + KC, cl] = c16[b][:, cols]
                    rf[base + 32 : base + 41, cl] = r1_all[b][:, cols]
        in_maps.append({"wf": wf, "rf": rf, "ge": gek})
    return in_maps


_NC_CACHE = {}


def _get_nc():
    if "nc" not in _NC_CACHE:
        _NC_CACHE["nc"] = build_nc()
    return _NC_CACHE["nc"]


def kernel(guidance, clusters, coords):
    guidance = np.asarray(guidance)
    clusters = np.asarray(clusters)
    coords = np.asarray(coords)
    in_maps = prepare_inputs(guidance, clusters, coords)
    nc = _get_nc()
    res = bass_utils.run_bass_kernel_spmd(nc, in_maps, list(range(NCORES)))
    # reassemble upper triangle, then mirror
    full = np.zeros((B, NS, NS), np.float32)
    for k in range(NCORES):
        o = res.results[k]["out"].astype(np.float32)  # [B, 128, CORE_COLS]
        for t, (X, j, n) in enumerate(_tiles_for_core(k)):
            rows = slice(128 * X, 128 * X + 128)
            cols = slice(128 * (X + j), 128 * (X + j) + 128 * n)
            full[:, rows, cols] = o[:, :, OFFS[t] : OFFS[t] + TW[t]]
    up = np.triu(full, 1)
    full = np.triu(full) + np.swapaxes(up, 1, 2)
    return full


# revision 24
# speedup vs baseline: 2.3483x; 1.0417x over previous
"""Trainium2 Bass kernel for nn_ContrastiveCRFLoss (self-contained).

Math: for each batch b and sample pair (n, m) over 2048 gathered pixels:
    out[b,n,m] = -(C[b,n,m] * (W1*exp(-cd - gd[b]/(2*BETA)) + W2*exp(-cd/(2*GAMMA))))
where C = cluster Gram, cd = squared coord distance, gd = squared guidance
distance.  The output is SYMMETRIC in (n, m): C is a Gram matrix and both
exp kernels depend on symmetric distances.

Device strategy (8 cores, upper-triangle only, mirrored on host):
  - The 2048x2048 pair grid is cut into 16 row-blocks of 128.  Block i needs
    only columns [128*i, 2048) (upper triangle incl. the diagonal block).
    Core k owns blocks k and 15-k: (16-k)+(k+1) = 17 col-units of 128 ->
    exactly 8 tiles of 256 cols + 1 tile of 128 cols per batch on EVERY
    core (SPMD-uniform shapes; per-core geometry lives in host staging).
  - Three fp16 matmul streams per tile, spread over all four PE row groups
    (tile_position 0/32/64/96) so independent streams run concurrently:
      even batches: Gram at rows 0,  exp1-arg at rows 32
      odd  batches: Gram at rows 64, exp1-arg at rows 96
      exp2-arg (batch-independent): rows 0 / 96 by chunk
    Exp arguments are produced directly in PSUM by augmenting the operands
    with ones/norm/log-weight rows (hi/lo fp16 splits keep this exact).
  - Inputs are packed so each parity loads with ONE [41 x cols] DMA (wc at
    group base, a1 at group base+32) - 6 input DMA triggers total.
  - Per 512-col chunk: ACT exp(PSUM)->fp16, add e1+e2 split GpSimd/DVE;
    DVE mult over 1024-col pC superchunks; one [128,2176] store per batch.
  - Host mirrors the strict upper triangle to reconstruct the full output.
"""

import numpy as np

import concourse.bass as bass
import concourse.mybir as mybir
import concourse.bass_utils as bass_utils
from concourse.tile import TileContext
from concourse.vector_clock import ScopedClock

F16 = mybir.dt.float16
F32 = mybir.dt.float32

# problem constants (hardcoded per the task contract)
ALPHA, BETA, GAMMA = 0.5, 0.15, 25.0
W1, W2, SHIFT = 10.0, 3.0, 0.0
B = 8
NS = 2048
NCORES = 8
KC, K1, K2 = 27, 9, 12
NT = 9                       # column tiles per batch per core
TW = [256] * 8 + [128]       # tile widths
# tile -> column offset in the 2176-wide per-batch strip.  Tiles 0-3 then
# tile 8 sit in the first 1152 cols (e2 group 0), tiles 4-7 in the last
# 1024 (e2 group 96).
OFFS = [0, 256, 512, 768, 1152, 1408, 1664, 1920, 1024]
CORE_COLS = 2176
CHUNKS = [(0, 512), (512, 512), (1024, 128), (1152, 512), (1664, 512)]
CHUNK_TILES = [(0, 1), (2, 3), (8,), (4, 5), (6, 7)]
G2E = [0, 0, 0, 96, 96]      # e2-phase row group by chunk
# pC/mult superchunks: (col offset, width, chunk indices)
SUPER = [(0, 1024, (0, 1)), (1024, 128, (2,)), (1152, 1024, (3, 4))]
# ge (e2 operand pack) column bookkeeping
WE_COL = {0: 0, 1: 128, 2: 256, 3: 384, 8: 512, 4: 1792, 5: 1920, 6: 2048, 7: 2176}
RE_BASE = {0: 640, 1: 1152, 2: 1664, 3: 2304, 4: 2816}
GE_COLS = 3328

# ---------------------------------------------------------------------------
# Walrus in this image rejects >1 sync wait per instruction. Split the Tile
# tail-drain's waits and any multi-wait instruction into single-wait NOPs.
# ---------------------------------------------------------------------------
_MAXW = 1


def _split_drain_and_barrier(self, tick_clock, wait_clock):
    probe = self.nc.sync.nop(nofuse=True)
    wait_clock.add_sem_waits(probe.ins, ScopedClock({None: tick_clock.global_clock}))
    si = probe.ins.sync_info
    waits = list(si.on_wait)
    probe.ins.sync_info = mybir.SyncInfo(
        on_wait=waits[:_MAXW], on_update=list(si.on_update)
    )
    for i in range(_MAXW, len(waits), _MAXW):
        n2 = self.nc.sync.nop(nofuse=True)
        n2.ins.sync_info = mybir.SyncInfo(on_wait=waits[i : i + _MAXW], on_update=[])
    self.nc.sync.drain()
    self.nc.all_engine_barrier()
    popped = self.nc._tile_sem_poison_stack.pop()
    assert popped is self._sem_poison
    self.nc.clear_and_free_semaphores(list(self.sems.allocated().values()))
    self.nc.all_engine_barrier()


def _split_multiwait_insts(nc):
    n_split = 0
    for fn in nc.m.functions:
        for bb in fn.blocks:
            insts = list(bb.instructions)
            new_insts = []
            changed = False
            for inst in insts:
                si = inst.sync_info
                waits = list(si.on_wait) if si is not None else []
                if len(waits) > _MAXW:
                    n_split += 1
                    changed = True
                    n_extra = len(waits) - _MAXW
                    for i in range(0, n_extra, _MAXW):
                        nop = mybir.InstNoOp(
                            name=nc.get_next_instruction_name(),
                            engine=inst.engine,
                            bass_nofuse=True,
                            sync_info=mybir.SyncInfo(
                                on_wait=waits[i : i + _MAXW], on_update=[]
                            ),
                        )
                        new_insts.append(nop)
                    inst.sync_info = mybir.SyncInfo(
                        on_wait=waits[n_extra:], on_update=list(si.on_update)
                    )
                new_insts.append(inst)
            if changed:
                bb.instructions = new_insts
    return n_split


def _install_tile_patch():
    TileContext._drain_and_barrier = _split_drain_and_barrier


def _tiles_for_core(k):
    """17 col-units as 8x256 + 1x128 tiles: (row_block, unit_start, n_units)."""
    WA, WB = 16 - k, k + 1
    A, Bb = k, 15 - k
    tiles = []
    for j in range(0, WA - 1, 2):
        tiles.append((A, j, 2))
    for j in range(0, WB - 1, 2):
        tiles.append((Bb, j, 2))
    if WA % 2 == 0:
        tiles.append((Bb, WB - 1, 1))
    else:
        tiles.append((A, WA - 1, 1))
    assert len(tiles) == NT and sum(n for _, _, n in tiles) == 17
    return tiles


# ---------------------------------------------------------------------------
# Device program (identical on all cores; data differs per core)
# ---------------------------------------------------------------------------

def build_nc():
    _install_tile_patch()
    nc = bass.Bass()
    WHALF = 4 * NT * 128   # 4608: per-parity lhsT slot columns
    RHALF = 4 * CORE_COLS  # 8704: per-parity rhs columns
    # Full-128-partition loads: DRAM->SBUF DMAs with few dest partitions
    # degrade to a single DMA engine (~27 GB/s); 128-partition transfers
    # stripe across all 16 engines like the stores do.  Even/odd parity
    # operands overlay the same columns at different partition groups.
    wf = nc.declare_dram_parameter("wf", [128, WHALF], F16, isOutput=False)
    rf = nc.declare_dram_parameter("rf", [128, RHALF], F16, isOutput=False)
    ge = nc.declare_dram_parameter("ge", [K2, GE_COLS], F16, isOutput=False)
    out = nc.declare_dram_parameter("out", [B, 128, CORE_COLS], F16, isOutput=True)

    with TileContext(nc) as tc:
        with (
            tc.tile_pool(name="w", bufs=1) as wpool,
            tc.tile_pool(name="r", bufs=1) as rpool,
            tc.tile_pool(name="e2p", bufs=1) as e2pool,
            tc.tile_pool(name="sb", bufs=12) as sbpool,
            tc.tile_pool(name="sm", bufs=4) as smpool,
            tc.tile_pool(name="ob", bufs=3) as opool,
            tc.tile_pool(name="psA", bufs=4, space="PSUM") as psa,
            tc.tile_pool(name="psB", bufs=2, space="PSUM") as psb,
        ):
            GE = wpool.tile([128, GE_COLS], F16)
            # per-batch-pair tiles: a reader only waits for its own load
            Rt = [rpool.tile([128, CORE_COLS], F16, name=f"R{i}") for i in range(4)]
            Wt = [wpool.tile([128, NT * 128], F16, name=f"W{i}") for i in range(4)]
            # e2 operands first (e2 phase unblocks earliest), then batch
            # pairs in order
            nc.sync.dma_start(GE[0:K2, 0:1792], ge[:, 0:1792])
            nc.sync.dma_start(GE[96 : 96 + K2, 1792:GE_COLS], ge[:, 1792:GE_COLS])
            for i in range(4):
                nc.sync.dma_start(
                    Rt[i][:], rf[:, i * CORE_COLS : (i + 1) * CORE_COLS]
                )
                nc.sync.dma_start(
                    Wt[i][:], wf[:, i * NT * 128 : (i + 1) * NT * 128]
                )

            e2 = e2pool.tile([128, CORE_COLS], F16)

            # --- e2 phase: batch-independent second-exp kernel ---
            for c, (off, wd) in enumerate(CHUNKS):
                g = G2E[c]
                p2 = psa.tile([128, 512], F32, tag="pA", name=f"p2c{c}")
                for t in CHUNK_TILES[c]:
                    w = TW[t]
                    toff = OFFS[t] - off
                    nc.tensor.matmul(
                        p2[:, toff : toff + w],
                        GE[g : g + K2, WE_COL[t] : WE_COL[t] + 128],
                        GE[g : g + K2, RE_BASE[c] + toff : RE_BASE[c] + toff + w],
                        start=True,
                        stop=True,
                        tile_position=(g, 0),
                    )
                nc.scalar.activation(
                    e2[:, off : off + wd],
                    p2[:, 0:wd],
                    mybir.ActivationFunctionType.Exp,
                )

            # --- batch loop ---
            for b in range(B):
                par = b % 2
                gc = 0 if par == 0 else 64
                g1 = 32 if par == 0 else 96
                W = Wt[b // 2]
                R = Rt[b // 2]
                wbase = 0
                rbase = 0
                ob = opool.tile([128, CORE_COLS], F16, tag="ob")
                gps_chunks = {0, 1, 2} if par == 0 else {0, 1, 3}
                e1s = {}
                for c, (off, wd) in enumerate(CHUNKS):
                    p1 = psa.tile([128, 512], F32, tag="pA", name=f"p1b{b}c{c}")
                    for t in CHUNK_TILES[c]:
                        w = TW[t]
                        toff = OFFS[t] - off
                        nc.tensor.matmul(
                            p1[:, toff : toff + w],
                            W[g1 : g1 + K1, wbase + t * 128 : wbase + (t + 1) * 128],
                            R[g1 : g1 + K1, rbase + OFFS[t] : rbase + OFFS[t] + w],
                            start=True,
                            stop=True,
                            tile_position=(g1, 0),
                        )
                    e1 = sbpool.tile([128, 512], F16, tag="e1")
                    nc.scalar.activation(
                        e1[:, 0:wd],
                        p1[:, 0:wd],
                        mybir.ActivationFunctionType.Exp,
                    )
                    e1s[c] = e1
                for so, swd, chs in SUPER:
                    pC = psb.tile([128, 1024], F32, tag="pB", name=f"pCb{b}o{so}")
                    s = smpool.tile([128, 1024], F16, tag="s")
                    for c in chs:
                        off, wd = CHUNKS[c]
                        for t in CHUNK_TILES[c]:
                            w = TW[t]
                            toff = OFFS[t] - so
                            nc.tensor.matmul(
                                pC[:, toff : toff + w],
                                W[gc : gc + KC, wbase + t * 128 : wbase + (t + 1) * 128],
                                R[gc : gc + KC, rbase + OFFS[t] : rbase + OFFS[t] + w],
                                start=True,
                                stop=True,
                                tile_position=(gc, 0),
                            )
                        soff = off - so
                        addfn = (
                            nc.gpsimd.tensor_add
                            if c in gps_chunks
                            else nc.vector.tensor_add
                        )
                        addfn(
                            s[:, soff : soff + wd],
                            e1s[c][:, 0:wd],
                            e2[:, off : off + wd],
                        )
                    nc.vector.tensor_tensor(
                        ob[:, so : so + swd],
                        pC[:, 0:swd],
                        s[:, 0:swd],
                        mybir.AluOpType.mult,
                    )
                nc.sync.dma_start(out[b], ob[:])

    _split_multiwait_insts(nc)
    return nc


# ---------------------------------------------------------------------------
# Host-side input prep
# ---------------------------------------------------------------------------

def _f16(x):
    return np.asarray(x, dtype=np.float16)


def _hi_lo(x):
    """Split fp64 vector into two fp16 rows summing to ~x."""
    hi = _f16(x)
    lo = _f16(x - hi.astype(np.float64))
    return hi, lo


def prepare_inputs(guidance, clusters, coords):
    ci = np.asarray(coords[0], dtype=np.int64)
    cj = np.asarray(coords[1], dtype=np.int64)
    sel_g = guidance[:, :, ci, cj].astype(np.float64)  # [B, 3, NS]
    sel_c = clusters[:, :, ci, cj].astype(np.float32)  # [B, 27, NS]

    # --- cluster Gram operands (fp16 snap) ---
    c16 = _f16(sel_c)
    wc_all = -c16  # lhsT (negated -> folds the leading minus)

    # --- first-exp argument operands: arg1 = -cd - gd/(2*beta) + ln(W1) ---
    u16 = _f16(sel_g / np.sqrt(2.0 * BETA))  # [B, 3, NS]
    xc16 = _f16(np.stack([ci, cj]) - 112.0)  # [2, NS] exact
    f1 = (u16.astype(np.float64) ** 2).sum(1) + (
        xc16.astype(np.float64) ** 2
    ).sum(0)  # [B, NS]
    a1_all = np.empty((B, K1, NS), np.float16)
    r1_all = np.empty((B, K1, NS), np.float16)
    ones = np.ones(NS, np.float16)
    for b in range(B):
        b1h, b1l = _hi_lo(np.log(W1) - f1[b])
        f1h, f1l = _hi_lo(f1[b])
        a1_all[b, 0:3] = u16[b]
        a1_all[b, 3:5] = xc16
        a1_all[b, 5] = ones
        a1_all[b, 6] = ones
        a1_all[b, 7] = f1h
        a1_all[b, 8] = f1l
        r1_all[b, 0:3] = _f16(2.0 * u16[b].astype(np.float64))
        r1_all[b, 3:5] = _f16(2.0 * xc16.astype(np.float64))
        r1_all[b, 5] = b1h
        r1_all[b, 6] = b1l
        r1_all[b, 7] = -ones
        r1_all[b, 8] = -ones

    # --- second-exp argument operands (batch independent) ---
    v = (np.stack([ci, cj]) - 112.0) / np.sqrt(2.0 * GAMMA)  # [2, NS]
    vh = _f16(v)
    vl = _f16(v - vh.astype(np.float64))
    vs = vh.astype(np.float64) + vl.astype(np.float64)
    f2 = (vs**2).sum(0)
    b2h, b2l = _hi_lo(np.log(W2) - f2)
    f2h, f2l = _hi_lo(f2)
    a2 = np.empty((K2, NS), np.float16)
    r2 = np.empty((K2, NS), np.float16)
    a2[0:2] = vh
    a2[2:4] = vh
    a2[4:6] = vl
    a2[6:8] = vl
    r2[0:2] = _f16(2.0 * vh.astype(np.float64))
    r2[2:4] = _f16(2.0 * vl.astype(np.float64))
    r2[4:6] = _f16(2.0 * vh.astype(np.float64))
    r2[6:8] = _f16(2.0 * vl.astype(np.float64))
    a2[8] = ones
    a2[9] = ones
    a2[10] = f2h
    a2[11] = f2l
    r2[8] = b2h
    r2[9] = b2l
    r2[10] = -ones
    r2[11] = -ones

    WHALF = 4 * NT * 128
    RHALF = 4 * CORE_COLS
    in_maps = []
    for k in range(NCORES):
        tiles = _tiles_for_core(k)
        wf = np.zeros((128, WHALF), np.float16)
        rf = np.zeros((128, RHALF), np.float16)
        gek = np.zeros((K2, GE_COLS), np.float16)
        for t, (X, j, n) in enumerate(tiles):
            rows = slice(128 * X, 128 * X + 128)
            cols = slice(128 * (X + j), 128 * (X + j) + 128 * n)
            o_t, w = OFFS[t], TW[t]
            gek[:, WE_COL[t] : WE_COL[t] + 128] = a2[:, rows]
            c = next(ci_ for ci_, ts in enumerate(CHUNK_TILES) if t in ts)
            toff = o_t - CHUNKS[c][0]
            gek[:, RE_BASE[c] + toff : RE_BASE[c] + toff + w] = r2[:, cols]
            for bi in range(4):
                for par, base in enumerate([0, 64]):
                    b = 2 * bi + par
                    sl = slice((bi * NT + t) * 128, (bi * NT + t + 1) * 128)
                    cl = slice(bi * CORE_COLS + o_t, bi * CORE_COLS + o_t + w)
                    wf[base : base + KC, sl] = wc_all[b][:, rows]
                    wf[base + 32 : base + 41, sl] = a1_all[b][:, rows]
                    rf[base : base + KC, cl] = c16[b][:, cols]
                    rf[base + 32 : base + 41, cl] = r1_all[b][:, cols]
        in_maps.append({"wf": wf, "rf": rf, "ge": gek})
    return in_maps


_NC_CACHE = {}


def _get_nc():
    if "nc" not in _NC_CACHE:
        _NC_CACHE["nc"] = build_nc()
    return _NC_CACHE["nc"]


def kernel(guidance, clusters, coords):
    guidance = np.asarray(guidance)
    clusters = np.asarray(clusters)
    coords = np.asarray(coords)
    in_maps = prepare_inputs(guidance, clusters, coords)
    nc = _get_nc()
    res = bass_utils.run_bass_kernel_spmd(nc, in_maps, list(range(NCORES)))
    # reassemble upper triangle, then mirror
    full = np.zeros((B, NS, NS), np.float32)
    for k in range(NCORES):
        o = res.results[k]["out"].astype(np.float32)  # [B, 128, CORE_COLS]
        for t, (X, j, n) in enumerate(_tiles_for_core(k)):
            rows = slice(128 * X, 128 * X + 128)
            cols = slice(128 * (X + j), 128 * (X + j) + 128 * n)
            full[:, rows, cols] = o[:, :, OFFS[t] : OFFS[t] + TW[t]]
    up = np.triu(full, 1)
    full = np.triu(full) + np.swapaxes(up, 1, 2)
    return full
